# revision 1
# baseline (speedup 1.0000x reference)
"""DyGraphGIN2d Trainium kernel: kNN graph (k=16) + GIN aggregation + MLP/BN/GELU.

Sharding: data-parallel over batch B=8 across 8 NeuronCores (one batch
element per core; kNN graphs are per-element independent). BatchNorm uses
training-mode batch statistics over ALL B*N rows, so the per-core partial
sums (a [64,2] tensor) go through one in-kernel AllReduce.

Algorithm per core (N=4096 points, C=64 dims):
  Phase 1 (threshold): ranking value s'[n,m] = <x_n,x_m> - |x_m|^2/2
    (distance-order-equivalent) is computed per 128-row stripe via three
    f32r matmuls: x = x_hi + x_lo is an EXACT split (f32r keeps 11
    explicit mantissa bits, so hi*hi/hi*lo/lo*hi products are exact and
    only lo*lo ~2^-24 is dropped) at 1 cy/row instead of fp32's 4 cy/row.
    DVE `max` (top-8) over 256-wide chunks + max/match_replace/max over
    the 128 candidates gives each row's exact 16th-largest value tau.
  Phase 2 (mask + aggregate): v[m,n] is recomputed in transposed
    orientation with the SAME mirrored 3-matmul sequence, making v ==
    s'.T bit-exact; mask = (v >= tau[n]) on DVE selects exactly the k
    nearest neighbors (self included). aggr[c,n] = x^T @ mask accumulates
    in PSUM via bf16 matmuls (mask is exactly 0/1 in bf16).
  Tail: h = (1+eps)x + aggr; h1 = W1^T h + b1; BN stats sum/sumsq ->
    AllReduce over 8 cores -> fused BN+erf-GELU in one ACT pass
    (gelu(h1*scale + shift) with per-partition scale/bias); out = W2^T hg
    + b2 -> DMA out in [O, N] layout per core.

The jitted 8-core shard_map executable is cached across kernel() calls.
"""

import numpy as np
import ml_dtypes

import concourse.bacc as bacc
import concourse.mybir as mybir
from concourse.tile import TileContext

F32 = mybir.dt.float32
F32R = mybir.dt.float32r
BF16 = mybir.dt.bfloat16
AF = mybir.ActivationFunctionType
ALU = mybir.AluOpType

B, C, N, O = 8, 64, 4096, 64
K_NN = 16
N_CORES = 8
NT = N // 128          # 32 row tiles
BN_EPS = 1e-5
BN_COUNT = float(B * N)

_cache = {}


def _build():
    nc = bacc.Bacc("TRN2", target_bir_lowering=False)

    xb_d = nc.dram_tensor("xb", [C, N], F32, kind="ExternalInput")
    id_d = nc.dram_tensor("ident64", [C, C], BF16, kind="ExternalInput")
    w1_d = nc.dram_tensor("w1", [C, O], F32, kind="ExternalInput")
    w2_d = nc.dram_tensor("w2", [O, O], F32, kind="ExternalInput")
    vecs_d = nc.dram_tensor("vecs", [O, 5], F32, kind="ExternalInput")  # b1,gamma,beta,b2,eps1
    ones_r_d = nc.dram_tensor("ones_row", [1, N], F32R, kind="ExternalInput")
    ones_c_d = nc.dram_tensor("ones_col", [C, 1], F32, kind="ExternalInput")
    y_d = nc.dram_tensor("y", [O, N], F32, kind="ExternalOutput")

    tau_dram_a = nc.dram_tensor("tau_scratch_a", [N // 2, 1], F32)  # internal
    tau_dram_b = nc.dram_tensor("tau_scratch_b", [N // 2, 1], F32)  # internal

    with TileContext(nc) as tc:
        with tc.tile_pool(name="big", bufs=1) as big, \
             tc.tile_pool(name="work", bufs=1) as work, \
             tc.tile_pool(name="dram", bufs=1, space="DRAM") as dpool:

            # ---- operand prep: exact hi/lo f32r decomposition ----------
            # f32r keeps 11 explicit mantissa bits; x = x_hi + x_lo with both
            # f32r-exact, so hi*hi + hi*lo + lo*hi are EXACT products (fp32
            # PSUM accumulate) and only lo*lo (~2^-24) is dropped. Three f32r
            # matmuls at 1 cy/row replace one fp32 matmul at 4 cy/row.
            # Ranking value: s' = <x_n, x_m> - sq_m/2 (distance-equivalent).
            # Aug rows: XH1 = [x_hi; 1], XH2 = [x_hi; q_hi], XLO = [x_lo; q_lo]
            # with q_hi + q_lo = -sq/2 split the same way.
            xb_sb = big.tile([64, N], F32)
            nc.sync.dma_start(xb_sb[:, :], xb_d[:, :])
            XH1 = big.tile([128, N], F32R)
            # XH2/XLO as 8 per-chunk column tiles so phase-1 matmuls on chunk
            # c only wait for chunk c's q-row (whole-tile deps otherwise
            # serialize ~30us of prep before the first stripe).
            XH2c = [big.tile([128, 512], F32R, name=f"XH2c{i}") for i in range(8)]
            XLOc = [big.tile([128, 512], F32R, name=f"XLOc{i}") for i in range(8)]
            for c8 in range(8):
                sl = slice(c8 * 512, (c8 + 1) * 512)
                nc.scalar.activation(XH1[:C, sl], xb_sb[:, sl], AF.Copy)
            nc.sync.dma_start(XH1[C : C + 1, :], ones_r_d[:, :])
            ones_col = work.tile([128, 1], F32)
            nc.sync.dma_start(ones_col[:C, :], ones_c_d[:, :])
            lo_t = work.tile([64, 512], F32, tag="lo_t", bufs=3)
            xsq = work.tile([64, 512], F32, tag="xsq", bufs=3)
            qtmp = work.tile([1, 512], F32, tag="qtmp", bufs=2)
            ps_qh = work
            with tc.tile_pool(name="ps_sq", bufs=2, space="PSUM") as ps_sq:
              for c8 in range(8):
                sl = slice(c8 * 512, (c8 + 1) * 512)
                nc.sync.dma_start(XH2c[c8][:C, :], XH1[:C, sl])
                nc.vector.tensor_tensor(out=lo_t[:, :], in0=xb_sb[:, sl],
                                        in1=XH1.bitcast(F32)[:C, sl],
                                        op=ALU.subtract)
                nc.vector.tensor_copy(XLOc[c8][:C, :], lo_t[:, :])
                nc.vector.tensor_tensor(out=xsq[:, :], in0=xb_sb[:, sl],
                                        in1=xb_sb[:, sl], op=ALU.mult)
                sq_ps = ps_sq.tile([1, 512], F32, tag="sq_ps")
                nc.tensor.matmul(sq_ps[:, :], ones_col[:C, :], xsq[:, :],
                                 start=True, stop=True)
                # q_hi straight into XH2 row 64 (ACT handles base-64 out),
                # plus a partition-0 twin for the exact q_lo subtraction.
                qh = ps_qh.tile([1, 512], F32R, tag="qh", bufs=2)
                nc.scalar.activation(XH2c[c8][C : C + 1, :], sq_ps[:, :], AF.Copy,
                                     scale=-0.5)
                nc.scalar.activation(qh[:, :], sq_ps[:, :], AF.Copy, scale=-0.5)
                nc.vector.tensor_scalar(out=qtmp[:, :], in0=sq_ps[:, :],
                                        scalar1=-0.5, scalar2=None, op0=ALU.mult)
                nc.vector.tensor_tensor(out=qtmp[:, :], in0=qtmp[:, :],
                                        in1=qh.bitcast(F32)[:, :],
                                        op=ALU.subtract)
                nc.scalar.activation(XLOc[c8][C : C + 1, :], qtmp[:, :], AF.Copy)

            # ---- phase 1: per-row 16th-largest threshold ----------------
            cand = work.tile([128, 128], F32, tag="cand", bufs=3)
            t8a = work.tile([128, 8], F32, tag="t8a", bufs=3)
            t8b = work.tile([128, 8], F32, tag="t8b", bufs=3)
            tau = work.tile([128, 1], F32, tag="tau", bufs=3)

            h_sb = big.tile([64, N], F32, tag="h_shared")  # assembled h
            eps1 = work.tile([64, 1], F32)
            nc.sync.dma_start(eps1[:, :], vecs_d[:, 4:5])
            tau_bc_a = big.tile([128, N // 2], F32)
            tau_bc_b = big.tile([128, N // 2], F32)
            mask_b = work.tile([128, 2048], BF16, tag="mask", bufs=3)

            ps_sv_cm = tc.tile_pool(name="ps_sv", bufs=4, space="PSUM")
            ps_sv = ps_sv_cm.__enter__()
            ps_s = ps_v = ps_sv
            _cms = [ps_sv_cm]

            def stripe(j):
                jj = slice(j * 128, (j + 1) * 128)
                for c8 in range(8):
                    s_ps = ps_s.tile([128, 512], F32, tag="sv_ps", name=f"s_ps_{j}_{c8}")
                    jt, jo = j // 4, (j % 4) * 128
                    nc.tensor.matmul(s_ps[:, :], XH1[: C + 1, jj],
                                     XH2c[c8][: C + 1, :], start=True, stop=False)
                    nc.tensor.matmul(s_ps[:, :], XH1[: C + 1, jj],
                                     XLOc[c8][: C + 1, :], start=False, stop=False)
                    nc.tensor.matmul(s_ps[:, :], XLOc[jt][:C, jo : jo + 128],
                                     XH2c[c8][:C, :], start=False, stop=True)
                    for h in range(2):
                        nc.vector.max(
                            out=cand[:, (c8 * 2 + h) * 8 : (c8 * 2 + h + 1) * 8],
                            in_=s_ps[:, h * 256 : (h + 1) * 256])
                nc.vector.max(out=t8a[:, :], in_=cand[:, :])
                nc.vector.match_replace(out=cand[:, :], in_to_replace=t8a[:, :],
                                        in_values=cand[:, :], imm_value=-1e30)
                nc.vector.max(out=t8b[:, :], in_=cand[:, :])
                # phase-2 recomputes s bit-exactly; tiny guard as insurance
                nc.vector.tensor_scalar(out=tau[:, :], in0=t8b[:, 7:8],
                                        scalar1=1e-6, scalar2=None,
                                        op0=ALU.subtract)
                if j < NT // 2:
                    nc.sync.dma_start(tau_dram_a[jj, :], tau[:, :])
                else:
                    nc.sync.dma_start(
                        tau_dram_b[(j - NT // 2) * 128 : (j - NT // 2 + 1) * 128, :],
                        tau[:, :])

            aggr_tiles = {}

            def p2_block(H, j):
                # one (H, j) unit: 8 f32r matmul thirds -> 4 masks -> 4 aggr
                aggr_ps = aggr_tiles[H]
                tb = tau_bc_a if H == 0 else tau_bc_b
                for c4 in range(4):
                    nsl = slice(H * 2048 + c4 * 512, H * 2048 + (c4 + 1) * 512)
                    v_ps = ps_v.tile([128, 512], F32, tag="sv_ps", name=f"v_ps_{H}_{j}_{c4}")
                    jt, jo = j // 4, (j % 4) * 128
                    nct = (H * 2048 + c4 * 512) // 512
                    nc.tensor.matmul(v_ps[:, :], XH2c[jt][: C + 1, jo : jo + 128],
                                     XH1[: C + 1, nsl], start=True, stop=False)
                    nc.tensor.matmul(v_ps[:, :], XLOc[jt][: C + 1, jo : jo + 128],
                                     XH1[: C + 1, nsl], start=False, stop=False)
                    nc.tensor.matmul(v_ps[:, :], XH2c[jt][:C, jo : jo + 128],
                                     XLOc[nct][:C, :], start=False, stop=True)
                    nc.vector.tensor_tensor(
                        out=mask_b[:, c4 * 512 : (c4 + 1) * 512],
                        in0=v_ps[:, :], in1=tb[:, c4 * 512 : (c4 + 1) * 512],
                        op=ALU.is_ge)
                for c4 in range(4):
                    nc.tensor.matmul(
                        aggr_ps[:, c4 * 512 : (c4 + 1) * 512],
                        xt_sb[:, j * C : (j + 1) * C],
                        mask_b[:, c4 * 512 : (c4 + 1) * 512],
                        start=(j == 0), stop=(j == NT - 1))

            def finish_half(H):
                hh = slice(H * 2048, (H + 1) * 2048)
                nc.vector.tensor_scalar(out=h_sb[:, hh], in0=xb_sb[:, hh],
                                        scalar1=eps1[:, :], scalar2=None, op0=ALU.mult)
                nc.vector.tensor_tensor(out=h_sb[:, hh], in0=h_sb[:, hh],
                                        in1=aggr_tiles[H][:, :], op=ALU.add)

            # stripes 0..15 (tau half a), then stripes 16..31 interleaved
            # with phase-2 H=0 blocks, then phase-2 H=1.
            for j in range(NT // 2):
                stripe(j)

            # xt16 (bf16 transposed x), emitted after the phase-1 prefix so
            # its PE/DVE stream slots don't delay the first stripes; copies
            # go to the otherwise-idle ACT engine.
            xb16 = work.tile([64, N], BF16)
            nc.vector.tensor_copy(xb16[:, :], xb_sb[:, :])
            ident = work.tile([64, C], BF16)
            nc.sync.dma_start(ident[:, :], id_d[:, :])
            xt_sb = work.tile([128, NT * C], BF16)
            with tc.tile_pool(name="ps_tp", bufs=2, space="PSUM") as ps_tp:
                for j in range(NT):
                    tp_ps = ps_tp.tile([128, C], BF16, tag="tp_ps")
                    nc.tensor.transpose(tp_ps[:, :],
                                        xb16[:, j * 128 : (j + 1) * 128],
                                        ident[:, :])
                    nc.scalar.activation(xt_sb[:, j * C : (j + 1) * C],
                                         tp_ps[:, :], AF.Copy)

            nc.sync.dma_start(
                tau_bc_a[:, :],
                tau_dram_a[:, 0:1].rearrange("m one -> one m").to_broadcast([128, N // 2]))
            ps_aggr_cm = tc.tile_pool(name="ps_aggr", bufs=1, space="PSUM")
            ps_aggr = ps_aggr_cm.__enter__()
            _cms.append(ps_aggr_cm)
            aggr_tiles[0] = ps_aggr.tile([64, 2048], F32, tag="aggr_ps", name="aggr0")
            for t in range(NT // 2):
                stripe(NT // 2 + t)
                p2_block(0, 2 * t)
                p2_block(0, 2 * t + 1)
            nc.sync.dma_start(
                tau_bc_b[:, :],
                tau_dram_b[:, 0:1].rearrange("m one -> one m").to_broadcast([128, N // 2]))
            finish_half(0)

            # W1 + BN partial stats for half 0, emitted inside the (PE-bound,
            # DVE/ACT-idle) H=1 segment so only half 1's stats + the
            # collective remain serial at the tail.
            w1_sb = work.tile([64, O], F32)
            w2_sb = work.tile([64, O], F32)
            vecs_sb = work.tile([64, 5], F32)
            nc.sync.dma_start(w1_sb[:, :], w1_d[:, :])
            nc.sync.dma_start(w2_sb[:, :], w2_d[:, :])
            nc.sync.dma_start(vecs_sb[:, :], vecs_d[:, :])
            h1_sb = big.tile([64, N], F32)
            sq_scratch = big.tile([64, N], F32, tag="hg_shared")
            stats_h = work.tile([64, 4], F32)  # cols: sum0, sumsq0, sum1, sumsq1

            def mlp_half(H):
                for c4 in range(4):
                    sl = slice(H * 2048 + c4 * 512, H * 2048 + (c4 + 1) * 512)
                    h1_ps = ps_v.tile([64, 512], F32, tag="sv_ps",
                                      name=f"h1_ps_{H}_{c4}")
                    nc.tensor.matmul(h1_ps[:, :], w1_sb[:, :], h_sb[:, sl],
                                     start=True, stop=True)
                    nc.vector.tensor_scalar(out=h1_sb[:, sl], in0=h1_ps[:, :],
                                            scalar1=vecs_sb[:, 0:1], scalar2=None,
                                            op0=ALU.add)
                hh = slice(H * 2048, (H + 1) * 2048)
                nc.vector.reduce_sum(stats_h[:, 2 * H : 2 * H + 1], h1_sb[:, hh],
                                     axis=mybir.AxisListType.X)
                nc.scalar.activation(sq_scratch[:, hh], h1_sb[:, hh], AF.Square,
                                     accum_out=stats_h[:, 2 * H + 1 : 2 * H + 2])

            aggr_tiles[1] = ps_aggr.tile([64, 2048], F32, tag="aggr_ps", name="aggr1")
            for j in range(NT // 2):
                p2_block(1, j)
            mlp_half(0)
            for j in range(NT // 2, NT):
                p2_block(1, j)
            finish_half(1)
            mlp_half(1)
            for cm in reversed(_cms):
                cm.__exit__(None, None, None)

            # ---- BN combine + GELU + W2 ---------------------------------
            ps_mlp_cm = tc.tile_pool(name="ps_mlp", bufs=4, space="PSUM")
            ps_mlp = ps_mlp_cm.__enter__()
            stats = work.tile([64, 2], F32)
            nc.vector.tensor_tensor(out=stats[:, :], in0=stats_h[:, 0:2],
                                    in1=stats_h[:, 2:4], op=ALU.add)

            cc_in = dpool.tile([64, 2], F32)
            cc_out = dpool.tile([64, 2], F32, addr_space="Shared")
            nc.sync.dma_start(cc_in[:, :], stats[:, :])
            nc.gpsimd.collective_compute(
                "AllReduce", ALU.add,
                ins=[cc_in[:, :]],
                outs=[cc_out[:, :]],
                replica_groups=[list(range(N_CORES))],
            )
            gstats = work.tile([64, 2], F32)
            nc.sync.dma_start(gstats[:, :], cc_out[:, :])

            # mean/var -> scale/shift  (all [64,1] minis)
            mean = work.tile([64, 1], F32)
            var = work.tile([64, 1], F32)
            scale = work.tile([64, 1], F32)
            shift = work.tile([64, 1], F32)
            tmp = work.tile([64, 1], F32)
            nc.vector.tensor_scalar(out=mean[:, :], in0=gstats[:, 0:1],
                                    scalar1=1.0 / BN_COUNT, scalar2=None, op0=ALU.mult)
            nc.vector.tensor_scalar(out=var[:, :], in0=gstats[:, 1:2],
                                    scalar1=1.0 / BN_COUNT, scalar2=None, op0=ALU.mult)
            nc.vector.tensor_tensor(out=tmp[:, :], in0=mean[:, :], in1=mean[:, :],
                                    op=ALU.mult)
            nc.vector.tensor_tensor(out=var[:, :], in0=var[:, :], in1=tmp[:, :],
                                    op=ALU.subtract)
            # rstd = 1/sqrt(var + eps)
            nc.vector.tensor_scalar(out=var[:, :], in0=var[:, :], scalar1=BN_EPS,
                                    scalar2=None, op0=ALU.add)
            nc.scalar.activation(tmp[:, :], var[:, :], AF.Sqrt)
            nc.vector.reciprocal(out=tmp[:, :], in_=tmp[:, :])
            nc.vector.tensor_tensor(out=scale[:, :], in0=vecs_sb[:, 1:2],
                                    in1=tmp[:, :], op=ALU.mult)  # gamma * rstd
            nc.vector.tensor_tensor(out=tmp[:, :], in0=mean[:, :], in1=scale[:, :],
                                    op=ALU.mult)
            nc.vector.tensor_tensor(out=shift[:, :], in0=vecs_sb[:, 2:3],
                                    in1=tmp[:, :], op=ALU.subtract)  # beta - mean*scale

            # fused BN + GELU on ACT: gelu(h1*scale + shift)
            hg = big.tile([64, N], F32, tag="hg_shared")
            nc.scalar.activation(hg[:, :], h1_sb[:, :], AF.Gelu,
                                 scale=scale[:, :], bias=shift[:, :])

            # out = w2^T hg + b2 -> y
            y_sb = big.tile([64, N], F32, tag="h_shared")
            for c8 in range(8):
                sl = slice(c8 * 512, (c8 + 1) * 512)
                o_ps = ps_mlp.tile([64, 512], F32, tag="h1_ps")
                nc.tensor.matmul(o_ps[:, :], w2_sb[:, :], hg[:, sl],
                                 start=True, stop=True)
                nc.vector.tensor_scalar(out=y_sb[:, sl], in0=o_ps[:, :],
                                        scalar1=vecs_sb[:, 3:4], scalar2=None,
                                        op0=ALU.add)
                nc.sync.dma_start(y_d[:, sl], y_sb[:, sl])
            ps_mlp_cm.__exit__(None, None, None)

    if not nc.is_finalized():
        nc.finalize()
    return nc


def _get_runner():
    """Build the Bass module once and cache a jitted 8-core executable.

    Mirrors bass2jax.run_bass_via_pjrt's multi-core path, but keeps the
    jitted shard_map callable across invocations (run_bass_via_pjrt
    rebuilds and retraces it per call, which costs hundreds of ms).
    """
    if "runner" in _cache:
        return _cache["runner"]

    import jax
    import concourse.mybir as mb
    from jax.sharding import Mesh, PartitionSpec
    from jax.experimental.shard_map import shard_map
    from concourse import bass2jax

    nc = _build()
    bass2jax.install_neuronx_cc_hook()

    partition_name = nc.partition_id_tensor.name if nc.partition_id_tensor else None
    in_names = []
    out_names = []
    out_avals = []
    for alloc in nc.m.functions[0].allocations:
        if not isinstance(alloc, mb.MemoryLocationSet):
            continue
        name = alloc.memorylocations[0].name
        if alloc.kind == "ExternalInput":
            if name != partition_name:
                in_names.append(name)
        elif alloc.kind == "ExternalOutput":
            out_names.append(name)
            out_avals.append(
                jax.core.ShapedArray(tuple(alloc.tensor_shape), mb.dt.np(alloc.dtype))
            )
    n_params = len(in_names)
    all_in_names = list(in_names)
    if partition_name is not None:
        all_in_names = all_in_names + [partition_name]

    def _body(*args):
        # No zero output operands: the kernel writes every output element,
        # so uninitialized custom-call result buffers are fine.
        operands = list(args)
        if partition_name is not None:
            operands.append(bass2jax.partition_id_tensor())
        outs = bass2jax._bass_exec_p.bind(
            *operands,
            out_avals=tuple(out_avals),
            in_names=tuple(all_in_names),
            out_names=tuple(out_names),
            lowering_input_output_aliases=(),
            sim_require_finite=True,
            sim_require_nnan=True,
            nc=nc,
        )
        return tuple(outs)

    devices = jax.devices()[:N_CORES]
    assert len(devices) == N_CORES, f"need {N_CORES} devices, have {len(jax.devices())}"
    mesh = Mesh(np.asarray(devices), ("core",))
    n_outs = len(out_names)
    sharded = jax.jit(
        shard_map(
            _body,
            mesh=mesh,
            in_specs=(PartitionSpec("core"),) * n_params,
            out_specs=(PartitionSpec("core"),) * n_outs,
            check_rep=False,
        ),
        keep_unused=True,
    )
    _cache["runner"] = (sharded, in_names, out_names, out_avals)
    return _cache["runner"]


def kernel(**inputs) -> np.ndarray:
    x = np.asarray(inputs["x"], dtype=np.float32)
    assert x.shape == (B, C, N, 1), x.shape
    k = int(np.asarray(inputs.get("k", K_NN)))
    assert k == K_NN, f"kernel compiled for k={K_NN}, got {k}"
    w1 = np.asarray(inputs["w1"], dtype=np.float32)
    b1 = np.asarray(inputs["b1"], dtype=np.float32)
    gamma = np.asarray(inputs["gamma"], dtype=np.float32)
    beta = np.asarray(inputs["beta"], dtype=np.float32)
    w2 = np.asarray(inputs["w2"], dtype=np.float32)
    b2 = np.asarray(inputs["b2"], dtype=np.float32)
    eps_gin = float(np.asarray(inputs["eps_gin"]))

    sharded, in_names, out_names, out_avals = _get_runner()

    xb = np.ascontiguousarray(x[:, :, :, 0])                     # [B, C, N]
    vecs = np.stack(
        [b1, gamma, beta, b2, np.full(O, 1.0 + eps_gin, np.float32)], axis=1
    ).astype(np.float32)                                         # [64, 5]
    ones_row = np.ones((1, N), np.float32)
    ones_col = np.ones((C, 1), np.float32)

    ident = np.eye(C, dtype=ml_dtypes.bfloat16)
    per_core = {
        "xb": xb,
        "ident64": np.broadcast_to(ident, (N_CORES,) + ident.shape),
        "w1": np.broadcast_to(w1, (N_CORES,) + w1.shape),
        "w2": np.broadcast_to(w2, (N_CORES,) + w2.shape),
        "vecs": np.broadcast_to(vecs, (N_CORES,) + vecs.shape),
        "ones_row": np.broadcast_to(ones_row, (N_CORES,) + ones_row.shape),
        "ones_col": np.broadcast_to(ones_col, (N_CORES,) + ones_col.shape),
    }
    # shard_map in_specs=P("core") take global arrays concatenated on axis 0
    concat_in = [
        np.ascontiguousarray(per_core[name]).reshape(
            (N_CORES * per_core[name].shape[1],) + per_core[name].shape[2:]
        )
        for name in in_names
    ]
    out_arrs = sharded(*concat_in)
    yi = out_names.index("y")
    y = np.asarray(out_arrs[yi]).reshape(N_CORES, O, N)
    return y[..., None].astype(np.float32)



# revision 13
# speedup vs baseline: 1.3212x; 1.3212x over previous
"""DyGraphGIN2d Trainium kernel: kNN graph (k=16) + GIN aggregation + MLP/BN/GELU.

Sharding: data-parallel over batch B=8 across 8 NeuronCores (one element per
core). BatchNorm batch statistics are combined with one small AllReduce.

Per-core algorithm (N=4096 nodes, C=64 channels). All static operand prep is
done HOST-side in numpy (f32r rounding, q = -|x|^2/2 split, bf16 transposed x,
(1+eps)x + rowsum/2) so the device runs only matmuls + scan + masks:

  Phase 1 (threshold): ranking value s[n,m] = <hi_n,hi_m> + q_hi_m + q_lo_m
    via ONE f32r matmul per [128,512] tile (66-row contraction: 64 hi rows +
    two ones rows picking up the exact q split; matmul cost depends only on
    columns). f32r operand rounding adds ~2.5e-3 noise to s, which flips the
    16/17-neighbor choice on ~0.3% of rows (measured end-to-end 5.7e-3 rel
    err vs the 2e-2 budget). DVE top-8 per 512-chunk -> 64 candidates ->
    max/match_replace/max gives each row's 16th-largest tau exactly.
  Phase 2 (mask+aggregate): v'[m,n] = s[n,m] - tau[n] recomputed in the
    transposed orientation with the same 66-row matmul plus a 67th row
    (-1 stationary x tau moving), bit-identical to phase 1 up to the final
    tau subtraction (guard 5e-5 covers its rounding). mask = Sign(v') on the
    ACT engine (+-1 exact in bf16, straight from PSUM - no DVE pass).
    aggr = xt^T @ mask accumulates 0.5*(sum_sel - sum_unsel) in PSUM;
    h = 0.5*aggr + [(1+eps)x + 0.5*rowsum] (host-prepped Xeps) on GPSIMD.
  Pipeline: column-slab c (512 n-cols) only needs tau from stripes 4c..4c+3,
    so mask/aggregate work for early slabs overlaps the DVE scan of later
    stripes (the scan, ~190us, is the pacing engine).
  Tail: h1 = w1^T h; BN stats sum/sumsq per slab (ACT accum) -> AllReduce
    -> fused BN+erf-GELU -> w2 -> y.
"""

import numpy as np
import ml_dtypes

import concourse.bacc as bacc
import concourse.mybir as mybir
from concourse.tile import TileContext

F32 = mybir.dt.float32
F32R = mybir.dt.float32r
BF16 = mybir.dt.bfloat16
AF = mybir.ActivationFunctionType
ALU = mybir.AluOpType

B, C, N, O = 8, 64, 4096, 64
K_NN = 16
N_CORES = 8
NT = N // 128            # 32 row stripes
NCH = N // 512           # 8 column chunks / slabs
BN_EPS = 1e-5
BN_COUNT = float(B * N)
TAU_GUARD = 5e-5

_cache = {}


def _f32r_round(a):
    """Round fp32 to 11 explicit mantissa bits (matches f32r storage)."""
    a = np.ascontiguousarray(a, np.float32)
    bits = a.view(np.uint32).astype(np.uint64)
    shift = 23 - 11
    half = np.uint64(1 << (shift - 1))
    mask = np.uint64(~((1 << shift) - 1) & 0xFFFFFFFF)
    return ((bits + half) & mask).astype(np.uint32).view(np.float32)


def _build():
    nc = bacc.Bacc("TRN2", target_bir_lowering=False)

    # host-prepped operands
    xh1_d = nc.dram_tensor("xh1", [66, N], F32R, kind="ExternalInput")   # hi;1;1
    xh2_d = nc.dram_tensor("xh2", [68, N], F32R, kind="ExternalInput")   # hi;qh;ql;-1;-1
    xt_d = nc.dram_tensor("xt", [128, NT * C], BF16, kind="ExternalInput")
    xeps_d = nc.dram_tensor("xeps", [C, N], F32, kind="ExternalInput")   # (1+e)x+rs/2
    w1_d = nc.dram_tensor("w1r", [C, O], F32, kind="ExternalInput")
    w2_d = nc.dram_tensor("w2r", [O, O], F32R, kind="ExternalInput")
    vecs_d = nc.dram_tensor("vecs", [O, 3], F32, kind="ExternalInput")   # gamma,beta,b2
    y_d = nc.dram_tensor("y", [O, N], F32, kind="ExternalOutput")
    tau_dram = nc.dram_tensor("tau_scratch", [N, 2], F32)                # internal

    with TileContext(nc) as tc:
        with tc.tile_pool(name="big", bufs=1) as big, \
             tc.tile_pool(name="work", bufs=1) as work, \
             tc.tile_pool(name="dram", bufs=1, space="DRAM") as dpool:

            # ---- inputs -> SBUF (chunked for fine-grained deps) ---------
            xh1c = [big.tile([68, 512], F32R, name=f"xh1c{i}") for i in range(NCH)]
            xh2c = [big.tile([68, 512], F32R, name=f"xh2c{i}") for i in range(NCH)]
            for i in range(NCH):
                sl = slice(i * 512, (i + 1) * 512)
                nc.sync.dma_start(xh1c[i][0:66, :], xh1_d[:, sl])
                nc.sync.dma_start(xh2c[i][:, :], xh2_d[:, sl])
            xt_sb = big.tile([128, NT * C], BF16)
            nc.sync.dma_start(xt_sb[:, :], xt_d[:, :])
            xeps_sb = big.tile([C, N], F32)
            nc.sync.dma_start(xeps_sb[:, :], xeps_d[:, :])
            w1_sb = work.tile([C, O], F32)
            w2_sb = work.tile([O, O], F32R)
            vecs_sb = work.tile([O, 3], F32)
            nc.sync.dma_start(w1_sb[:, :], w1_d[:, :])
            nc.sync.dma_start(w2_sb[:, :], w2_d[:, :])
            nc.sync.dma_start(vecs_sb[:, :], vecs_d[:, :])

            h_sb = big.tile([C, N], F32)
            h1_sb = big.tile([C, N], F32)
            sq_scr = big.tile([C, N], F32)
            hg_sb = big.tile([C, N], F32R)
            y_sb = big.tile([C, N], F32)
            stats_s = work.tile([O, NCH], F32)   # per-slab sum(h1)
            stats_q = work.tile([O, NCH], F32)   # per-slab sum(h1^2)

            cand = work.tile([128, 64], F32, tag="cand", bufs=3)
            t8a = work.tile([128, 8], F32, tag="t8a", bufs=3)
            t8b = work.tile([128, 8], F32, tag="t8b", bufs=3)
            tau_f = work.tile([128, 1], F32, tag="tauf", bufs=3)
            tau2 = work.tile([128, 2], F32R, tag="tau2", bufs=3)
            masks = work.tile([128, 512], BF16, tag="mask", bufs=6)

            ps_s_cm = tc.tile_pool(name="ps_s", bufs=2, space="PSUM")
            ps_v_cm = tc.tile_pool(name="ps_v", bufs=2, space="PSUM")
            ps_a_cm = tc.tile_pool(name="ps_a", bufs=2, space="PSUM")
            ps_m_cm = tc.tile_pool(name="ps_m", bufs=2, space="PSUM")
            ps_s = ps_s_cm.__enter__()
            ps_v = ps_v_cm.__enter__()
            ps_a = ps_a_cm.__enter__()
            ps_m = ps_m_cm.__enter__()
            _cms = [ps_s_cm, ps_v_cm, ps_a_cm, ps_m_cm]

            aggr_tiles = {}

            def stripe(j):
                """Phase-1: s[n in stripe j, :] + DVE 16th-largest -> tau DMA."""
                jt, jo = j // 4, (j % 4) * 128
                for c in range(NCH):
                    s_ps = ps_s.tile([128, 512], F32, tag="s_ps",
                                     name=f"s_{j}_{c}")
                    nc.tensor.matmul(s_ps[:, :], xh1c[jt][0:66, jo:jo + 128],
                                     xh2c[c][0:66, :], start=True, stop=True)
                    nc.vector.max(out=cand[:, c * 8:(c + 1) * 8], in_=s_ps[:, :])
                nc.vector.max(out=t8a[:, :], in_=cand[:, :])
                nc.vector.match_replace(out=cand[:, :], in_to_replace=t8a[:, :],
                                        in_values=cand[:, :], imm_value=-1e30)
                nc.vector.max(out=t8b[:, :], in_=cand[:, :])
                # tau = t16 - guard, split exactly into f32r hi + lo rows so
                # the phase-2 matmul subtracts it at full fp32 precision.
                nc.vector.tensor_scalar(out=tau_f[:, :], in0=t8b[:, 7:8],
                                        scalar1=TAU_GUARD, scalar2=None,
                                        op0=ALU.subtract)
                nc.scalar.activation(tau2[:, 0:1], tau_f[:, :], AF.Copy)
                nc.gpsimd.tensor_tensor(out=tau2.bitcast(F32)[:, 1:2],
                                        in0=tau_f[:, :],
                                        in1=tau2.bitcast(F32)[:, 0:1],
                                        op=ALU.subtract)
                nc.sync.dma_start(tau_dram[j * 128:(j + 1) * 128, :],
                                  tau2.bitcast(F32)[:, :])

            def tau_load(c):
                """tau hi/lo rows for slab c into xh1c[c] partitions 66/67."""
                for r in range(2):
                    nc.sync.dma_start(
                        xh1c[c].bitcast(F32)[66 + r:67 + r, :],
                        tau_dram[c * 512:(c + 1) * 512, r:r + 1]
                        .rearrange("m one -> one m"))

            def unit(c, j):
                """Phase-2 unit: v'[stripe j, slab c] -> Sign mask -> aggr mm."""
                jt, jo = j // 4, (j % 4) * 128
                v_ps = ps_v.tile([128, 512], F32, tag="v_ps", name=f"v_{c}_{j}")
                nc.tensor.matmul(v_ps[:, :], xh2c[jt][0:68, jo:jo + 128],
                                 xh1c[c][0:68, :], start=True, stop=True)
                nc.scalar.activation(masks[:, :], v_ps[:, :], AF.Sign)
                nc.tensor.matmul(aggr_tiles[c][:, :],
                                 xt_sb[:, j * C:(j + 1) * C], masks[:, :],
                                 start=(j == 0), stop=(j == NT - 1))

            aggr_sb = work.tile([O, 512], F32, tag="aggr_sb", bufs=2)

            def post_slab(c):
                """h = 0.5*aggr + Xeps; h1 = w1^T h; BN partial stats."""
                sl = slice(c * 512, (c + 1) * 512)
                nc.scalar.activation(aggr_sb[:, :], aggr_tiles[c][:, :], AF.Copy)
                nc.gpsimd.tensor_tensor(out=h_sb[:, sl], in0=aggr_sb[:, :],
                                        in1=xeps_sb[:, sl], op=ALU.add)
                h1_ps = ps_m.tile([O, 512], F32, tag="h1_ps", name=f"h1_{c}")
                nc.tensor.matmul(h1_ps[:, :], w1_sb[:, :], h_sb[:, sl],
                                 start=True, stop=True)
                nc.scalar.activation(h1_sb[:, sl], h1_ps[:, :], AF.Copy,
                                     accum_out=stats_s[:, c:c + 1])
                nc.scalar.activation(sq_scr[:, sl], h1_sb[:, sl], AF.Square,
                                     accum_out=stats_q[:, c:c + 1])

            # ---- emission: scan-paced interleave ------------------------
            def make_aggr(c):
                aggr_tiles[c] = ps_a.tile([O, 512], F32, tag="aggr", name=f"ag{c}")

            # stripes 0..3 first (tau chunk 0), then one stripe + 8 units per
            # round; slab c's units land in rounds 4c..4c+3 by construction.
            for j in range(4):
                stripe(j)
            tau_load(0)
            uq = [(c, j) for c in range(NCH - 1) for j in range(NT)]
            for t in range(28):
                stripe(4 + t)
                if t % 4 == 3 and t // 4 + 1 < NCH:
                    tau_load(t // 4 + 1)
                for (c, j) in uq[t * 8:(t + 1) * 8]:
                    if j == 0:
                        make_aggr(c)
                    unit(c, j)
                    if j == NT - 1:
                        post_slab(c)
            for j in range(NT):
                if j == 0:
                    make_aggr(NCH - 1)
                unit(NCH - 1, j)
            post_slab(NCH - 1)

            # ---- BN combine + AllReduce + GELU + W2 ---------------------
            stats = work.tile([O, 2], F32)
            nc.vector.reduce_sum(stats[:, 0:1], stats_s[:, :],
                                 axis=mybir.AxisListType.X)
            nc.vector.reduce_sum(stats[:, 1:2], stats_q[:, :],
                                 axis=mybir.AxisListType.X)

            cc_in = dpool.tile([O, 2], F32)
            cc_out = dpool.tile([O, 2], F32, addr_space="Shared")
            nc.sync.dma_start(cc_in[:, :], stats[:, :])
            nc.gpsimd.collective_compute(
                "AllReduce", ALU.add,
                ins=[cc_in[:, :]],
                outs=[cc_out[:, :]],
                replica_groups=[list(range(N_CORES))],
            )
            gstats = work.tile([O, 2], F32)
            nc.sync.dma_start(gstats[:, :], cc_out[:, :])

            mean = work.tile([O, 1], F32)
            var = work.tile([O, 1], F32)
            scale = work.tile([O, 1], F32)
            shift = work.tile([O, 1], F32)
            tmp = work.tile([O, 1], F32)
            nc.vector.tensor_scalar(out=mean[:, :], in0=gstats[:, 0:1],
                                    scalar1=1.0 / BN_COUNT, scalar2=None,
                                    op0=ALU.mult)
            nc.vector.tensor_scalar(out=var[:, :], in0=gstats[:, 1:2],
                                    scalar1=1.0 / BN_COUNT, scalar2=None,
                                    op0=ALU.mult)
            nc.vector.tensor_tensor(out=tmp[:, :], in0=mean[:, :], in1=mean[:, :],
                                    op=ALU.mult)
            nc.vector.tensor_tensor(out=var[:, :], in0=var[:, :], in1=tmp[:, :],
                                    op=ALU.subtract)
            nc.vector.tensor_scalar(out=var[:, :], in0=var[:, :], scalar1=BN_EPS,
                                    scalar2=None, op0=ALU.add)
            nc.scalar.activation(tmp[:, :], var[:, :], AF.Sqrt)
            nc.vector.reciprocal(out=tmp[:, :], in_=tmp[:, :])
            nc.vector.tensor_tensor(out=scale[:, :], in0=vecs_sb[:, 0:1],
                                    in1=tmp[:, :], op=ALU.mult)
            nc.vector.tensor_tensor(out=tmp[:, :], in0=mean[:, :], in1=scale[:, :],
                                    op=ALU.mult)
            nc.vector.tensor_tensor(out=shift[:, :], in0=vecs_sb[:, 1:2],
                                    in1=tmp[:, :], op=ALU.subtract)

            for c in range(NCH):
                sl = slice(c * 512, (c + 1) * 512)
                nc.scalar.activation(hg_sb[:, sl], h1_sb[:, sl], AF.Gelu,
                                     scale=scale[:, :], bias=shift[:, :])
                o_ps = ps_m.tile([O, 512], F32, tag="h1_ps", name=f"o_{c}")
                nc.tensor.matmul(o_ps[:, :], w2_sb[:, :], hg_sb[:, sl],
                                 start=True, stop=True)
                nc.vector.tensor_scalar(out=y_sb[:, sl], in0=o_ps[:, :],
                                        scalar1=vecs_sb[:, 2:3], scalar2=None,
                                        op0=ALU.add)
                nc.sync.dma_start(y_d[:, sl], y_sb[:, sl])

            for cm in reversed(_cms):
                cm.__exit__(None, None, None)

    if not nc.is_finalized():
        nc.finalize()
    return nc


def _get_runner():
    """Build once; cache a jitted 8-core shard_map executable."""
    if "runner" in _cache:
        return _cache["runner"]

    import jax
    import concourse.mybir as mb
    from jax.sharding import Mesh, PartitionSpec
    from jax.experimental.shard_map import shard_map
    from concourse import bass2jax

    nc = _build()
    bass2jax.install_neuronx_cc_hook()

    partition_name = nc.partition_id_tensor.name if nc.partition_id_tensor else None
    in_names = []
    out_names = []
    out_avals = []
    for alloc in nc.m.functions[0].allocations:
        if not isinstance(alloc, mb.MemoryLocationSet):
            continue
        name = alloc.memorylocations[0].name
        if alloc.kind == "ExternalInput":
            if name != partition_name:
                in_names.append(name)
        elif alloc.kind == "ExternalOutput":
            out_names.append(name)
            out_avals.append(
                jax.core.ShapedArray(tuple(alloc.tensor_shape), mb.dt.np(alloc.dtype))
            )
    n_params = len(in_names)
    all_in_names = list(in_names)
    if partition_name is not None:
        all_in_names = all_in_names + [partition_name]

    def _body(*args):
        operands = list(args)
        if partition_name is not None:
            operands.append(bass2jax.partition_id_tensor())
        outs = bass2jax._bass_exec_p.bind(
            *operands,
            out_avals=tuple(out_avals),
            in_names=tuple(all_in_names),
            out_names=tuple(out_names),
            lowering_input_output_aliases=(),
            sim_require_finite=True,
            sim_require_nnan=True,
            nc=nc,
        )
        return tuple(outs)

    devices = jax.devices()[:N_CORES]
    assert len(devices) == N_CORES, f"need {N_CORES} devices, have {len(jax.devices())}"
    mesh = Mesh(np.asarray(devices), ("core",))
    n_outs = len(out_names)
    sharded = jax.jit(
        shard_map(
            _body,
            mesh=mesh,
            in_specs=(PartitionSpec("core"),) * n_params,
            out_specs=(PartitionSpec("core"),) * n_outs,
            check_rep=False,
        ),
        keep_unused=True,
    )
    _cache["runner"] = (sharded, in_names, out_names, out_avals)
    return _cache["runner"]


def kernel(**inputs) -> np.ndarray:
    x = np.asarray(inputs["x"], dtype=np.float32)
    assert x.shape == (B, C, N, 1), x.shape
    k = int(np.asarray(inputs.get("k", K_NN)))
    assert k == K_NN, f"kernel compiled for k={K_NN}, got {k}"
    w1 = np.asarray(inputs["w1"], dtype=np.float32)
    b1 = np.asarray(inputs["b1"], dtype=np.float32)  # cancels through BN stats
    gamma = np.asarray(inputs["gamma"], dtype=np.float32)
    beta = np.asarray(inputs["beta"], dtype=np.float32)
    w2 = np.asarray(inputs["w2"], dtype=np.float32)
    b2 = np.asarray(inputs["b2"], dtype=np.float32)
    eps_gin = float(np.asarray(inputs["eps_gin"]))
    del b1

    sharded, in_names, out_names, out_avals = _get_runner()

    xb = np.ascontiguousarray(x[:, :, :, 0])                 # [B, C, N]
    hi = _f32r_round(xb)                                     # [B, C, N]
    sq = (xb.astype(np.float64) ** 2).sum(axis=1)            # [B, N]
    q_hi = _f32r_round((-0.5 * sq).astype(np.float32))
    q_lo = _f32r_round((-0.5 * sq - q_hi.astype(np.float64)).astype(np.float32))

    xh1 = np.empty((B, 66, N), np.float32)
    xh1[:, :C] = hi
    xh1[:, C] = 1.0
    xh1[:, C + 1] = 1.0
    xh2 = np.empty((B, 68, N), np.float32)
    xh2[:, :C] = hi
    xh2[:, C] = q_hi
    xh2[:, C + 1] = q_lo
    xh2[:, C + 2] = -1.0
    xh2[:, C + 3] = -1.0

    xt16 = xb.astype(ml_dtypes.bfloat16)                     # [B, C, N]
    # xt[p, j*C + c] = 0.5 * bf16(x[c, j*128 + p])  (halved exactly, so the
    # +-1 sign-mask aggregation lands as 0.5*S_sign in PSUM)
    xt_half = (xt16.astype(np.float32) * 0.5).astype(ml_dtypes.bfloat16)
    xt = np.ascontiguousarray(
        xt_half.reshape(B, C, NT, 128).transpose(0, 3, 2, 1).reshape(B, 128, NT * C))
    rowsum = xt16.astype(np.float64).sum(axis=2)             # [B, C]
    xeps = ((1.0 + eps_gin) * xb.astype(np.float64)
            + 0.5 * rowsum[:, :, None]).astype(np.float32)   # [B, C, N]

    vecs = np.stack([gamma, beta, b2], axis=1).astype(np.float32)
    per_core = {
        "xh1": xh1,
        "xh2": xh2,
        "xt": xt,
        "xeps": xeps,
        "w1r": np.broadcast_to(w1, (N_CORES,) + w1.shape),
        "w2r": np.broadcast_to(_f32r_round(w2), (N_CORES,) + w2.shape),
        "vecs": np.broadcast_to(vecs, (N_CORES,) + vecs.shape),
    }
    concat_in = [
        np.ascontiguousarray(per_core[name]).reshape(
            (N_CORES * per_core[name].shape[1],) + per_core[name].shape[2:]
        )
        for name in in_names
    ]
    out_arrs = sharded(*concat_in)
    yi = out_names.index("y")
    y = np.asarray(out_arrs[yi]).reshape(N_CORES, O, N)
    return y[..., None].astype(np.float32)


# revision 22
# speedup vs baseline: 1.8611x; 1.4086x over previous
"""DyGraphGIN2d Trainium kernel: kNN graph (k=16) + GIN aggregation + MLP/BN/GELU.

Sharding: data-parallel over batch B=8 across 8 NeuronCores (one element per
core). BatchNorm batch statistics are combined with one small AllReduce.

Per-core algorithm (N=4096 nodes, C=64 channels). All static operand prep is
done HOST-side in numpy (f32r rounding, q = -|x|^2/2 split, bf16 transposed x,
(1+eps)x + rowsum/2) so the device runs only matmuls + scan + masks:

  Phase 1 (threshold): ranking value s[n,m] = <hi_n,hi_m> + q_hi_m + q_lo_m
    via ONE f32r matmul per [128,512] tile (66-row contraction: 64 hi rows +
    two ones rows picking up the exact q split; matmul cost depends only on
    columns). f32r operand rounding adds ~2.5e-3 noise to s, which flips the
    16/17-neighbor choice on ~0.3% of rows (measured end-to-end 5.7e-3 rel
    err vs the 2e-2 budget). DVE top-8 per 512-chunk -> 64 candidates ->
    max/match_replace/max gives each row's 16th-largest tau exactly.
  Phase 2 (mask+aggregate): v'[m,n] = s[n,m] - tau[n] recomputed in the
    transposed orientation with the same 66-row matmul plus a 67th row
    (-1 stationary x tau moving), bit-identical to phase 1 up to the final
    tau subtraction (guard 5e-5 covers its rounding). mask = Sign(v') on the
    ACT engine (+-1 exact in bf16, straight from PSUM - no DVE pass).
    aggr = xt^T @ mask accumulates 0.5*(sum_sel - sum_unsel) in PSUM;
    h = 0.5*aggr + [(1+eps)x + 0.5*rowsum] (host-prepped Xeps) on GPSIMD.
  Pipeline: column-slab c (512 n-cols) only needs tau from stripes 4c..4c+3,
    so mask/aggregate work for early slabs overlaps the DVE scan of later
    stripes (the scan, ~190us, is the pacing engine).
  Tail: h1 = w1^T h; BN stats sum/sumsq per slab (ACT accum) -> AllReduce
    -> fused BN+erf-GELU -> w2 -> y.
"""

import numpy as np
import ml_dtypes

import concourse.bacc as bacc
import concourse.mybir as mybir
from concourse.tile import TileContext

F32 = mybir.dt.float32
F32R = mybir.dt.float32r
BF16 = mybir.dt.bfloat16
AF = mybir.ActivationFunctionType
ALU = mybir.AluOpType

B, C, N, O = 8, 64, 4096, 64
K_NN = 16
N_CORES = 8
NT = N // 128            # 32 row stripes
NCH = N // 512           # 8 column chunks / slabs
BN_EPS = 1e-5
BN_COUNT = float(B * N)
TAU_GUARD = 5e-5

_cache = {}


def _f32r_round(a):
    """Round fp32 to 11 explicit mantissa bits (matches f32r storage)."""
    a = np.ascontiguousarray(a, np.float32)
    bits = a.view(np.uint32).astype(np.uint64)
    shift = 23 - 11
    half = np.uint64(1 << (shift - 1))
    mask = np.uint64(~((1 << shift) - 1) & 0xFFFFFFFF)
    return ((bits + half) & mask).astype(np.uint32).view(np.float32)


def _build():
    nc = bacc.Bacc("TRN2", target_bir_lowering=False)

    # host-prepped operands
    xh1_d = nc.dram_tensor("xh1", [66, N], F32R, kind="ExternalInput")   # hi;1;1
    xh2_d = nc.dram_tensor("xh2", [68, N], F32R, kind="ExternalInput")   # hi;qh;ql;-1;-1
    xt_d = nc.dram_tensor("xt", [128, NT * C], BF16, kind="ExternalInput")
    xeps_d = nc.dram_tensor("xeps", [C, N], F32, kind="ExternalInput")   # (1+e)x+rs/2
    w1_d = nc.dram_tensor("w1r", [C, O], F32, kind="ExternalInput")
    w2_d = nc.dram_tensor("w2r", [O, O], F32R, kind="ExternalInput")
    vecs_d = nc.dram_tensor("vecs", [O, 3], F32, kind="ExternalInput")   # gamma,beta,b2
    y_d = nc.dram_tensor("y", [O, N], F32, kind="ExternalOutput")
    tau_dram = nc.dram_tensor("tau_scratch", [N, 2], F32)                # internal

    with TileContext(nc) as tc:
        with tc.tile_pool(name="big", bufs=1) as big, \
             tc.tile_pool(name="work", bufs=1) as work, \
             tc.tile_pool(name="dram", bufs=1, space="DRAM") as dpool:

            # ---- inputs -> SBUF (chunked for fine-grained deps) ---------
            xh1c = [big.tile([68, 512], F32R, name=f"xh1c{i}") for i in range(NCH)]
            xh2c = [big.tile([68, 512], F32R, name=f"xh2c{i}") for i in range(NCH)]
            for i in range(NCH):
                sl = slice(i * 512, (i + 1) * 512)
                nc.sync.dma_start(xh1c[i][0:66, :], xh1_d[:, sl])
                nc.sync.dma_start(xh2c[i][:, :], xh2_d[:, sl])
            xt_sb = big.tile([128, NT * C], BF16)
            nc.sync.dma_start(xt_sb[:, :], xt_d[:, :])
            xeps_sb = big.tile([C, N], F32)
            nc.sync.dma_start(xeps_sb[:, :], xeps_d[:, :])
            w1_sb = work.tile([C, O], F32)
            w2_sb = work.tile([O, O], F32R)
            vecs_sb = work.tile([O, 3], F32)
            nc.sync.dma_start(w1_sb[:, :], w1_d[:, :])
            nc.sync.dma_start(w2_sb[:, :], w2_d[:, :])
            nc.sync.dma_start(vecs_sb[:, :], vecs_d[:, :])

            h_sb = big.tile([C, N], F32)
            h1_sb = big.tile([C, N], F32)
            sq_scr = big.tile([C, N], F32)
            hg_sb = big.tile([C, N], F32R)
            y_sb = big.tile([C, N], F32)
            stats_s = work.tile([O, NCH], F32)   # per-slab sum(h1)
            stats_q = work.tile([O, NCH], F32)   # per-slab sum(h1^2)

            cand = work.tile([128, 64], F32, tag="cand", bufs=3)
            t8a = work.tile([128, 8], F32, tag="t8a", bufs=3)
            t8b = work.tile([128, 8], F32, tag="t8b", bufs=3)
            tau_f = work.tile([128, 1], F32, tag="tauf", bufs=3)
            tau2 = work.tile([128, 2], F32R, tag="tau2", bufs=3)
            NMASK = 8
            mask_ring = [work.tile([128, 512], BF16, name=f"mask{i}")
                         for i in range(NMASK)]

            ps_s_cm = tc.tile_pool(name="ps_s", bufs=2, space="PSUM")
            ps_v_cm = tc.tile_pool(name="ps_v", bufs=3, space="PSUM")
            ps_a_cm = tc.tile_pool(name="ps_a", bufs=2, space="PSUM")
            ps_m_cm = tc.tile_pool(name="ps_m", bufs=1, space="PSUM")
            ps_s = ps_s_cm.__enter__()
            ps_v = ps_v_cm.__enter__()
            ps_a = ps_a_cm.__enter__()
            ps_m = ps_m_cm.__enter__()
            _cms = [ps_s_cm, ps_v_cm, ps_a_cm, ps_m_cm]

            aggr_tiles = {}

            def s_piece(j, c):
                """One phase-1 s-matmul chunk + its DVE top-8."""
                jt, jo = j // 4, (j % 4) * 128
                s_ps = ps_s.tile([128, 512], F32, tag="s_ps", name=f"s_{j}_{c}")
                nc.tensor.matmul(s_ps[:, :], xh1c[jt][0:66, jo:jo + 128],
                                 xh2c[c][0:66, :], start=True, stop=True)
                nc.vector.max(out=cand[:, c * 8:(c + 1) * 8], in_=s_ps[:, :])

            def stripe_merge(j):
                """DVE merge of the 64 candidates -> tau hi/lo -> DMA."""
                nc.vector.max(out=t8a[:, :], in_=cand[:, :])
                nc.vector.match_replace(out=cand[:, :], in_to_replace=t8a[:, :],
                                        in_values=cand[:, :], imm_value=-1e30)
                nc.vector.max(out=t8b[:, :], in_=cand[:, :])
                # tau = t16 - guard, split exactly into f32r hi + lo rows so
                # the phase-2 matmul subtracts it at full fp32 precision.
                nc.vector.tensor_scalar(out=tau_f[:, :], in0=t8b[:, 7:8],
                                        scalar1=TAU_GUARD, scalar2=None,
                                        op0=ALU.subtract)
                nc.vector.tensor_copy(tau2[:, 0:1], tau_f[:, :])
                nc.gpsimd.tensor_tensor(out=tau2.bitcast(F32)[:, 1:2],
                                        in0=tau_f[:, :],
                                        in1=tau2.bitcast(F32)[:, 0:1],
                                        op=ALU.subtract)
                nc.sync.dma_start(tau_dram[j * 128:(j + 1) * 128, :],
                                  tau2.bitcast(F32)[:, :])

            def tau_load(c):
                """tau hi/lo rows for slab c into xh1c[c] partitions 66/67."""
                for r in range(2):
                    nc.sync.dma_start(
                        xh1c[c].bitcast(F32)[66 + r:67 + r, :],
                        tau_dram[c * 512:(c + 1) * 512, r:r + 1]
                        .rearrange("m one -> one m"))

            _ucount = [0]

            def unit_v(c, j):
                """Phase-2: v'[stripe j, slab c] matmul -> ACT Sign mask."""
                g = _ucount[0]
                _ucount[0] += 1
                jt, jo = j // 4, (j % 4) * 128
                v_ps = ps_v.tile([128, 512], F32, tag="v_ps", name=f"v_{c}_{j}")
                nc.tensor.matmul(v_ps[:, :], xh2c[jt][0:68, jo:jo + 128],
                                 xh1c[c][0:68, :], start=True, stop=True)
                mt = mask_ring[g % NMASK]
                nc.scalar.activation(mt[:, :], v_ps[:, :], AF.Sign)
                return (c, j, mt)

            def unit_a(cjm):
                """Lagged aggregation matmul for a completed mask."""
                c, j, mt = cjm
                nc.tensor.matmul(aggr_tiles[c][:, :],
                                 xt_sb[:, j * C:(j + 1) * C], mt[:, :],
                                 start=(j == 0), stop=(j == NT - 1))

            aggr_sb = work.tile([O, 512], F32, tag="aggr_sb", bufs=2)

            def post_slab_a(c):
                """h = 0.5*aggr + Xeps (ACT copy out of PSUM, Pool add), w1."""
                sl = slice(c * 512, (c + 1) * 512)
                nc.scalar.activation(aggr_sb[:, :], aggr_tiles[c][:, :], AF.Copy)
                nc.gpsimd.tensor_tensor(out=h_sb[:, sl], in0=aggr_sb[:, :],
                                        in1=xeps_sb[:, sl], op=ALU.add)
                h1_ps = ps_m.tile([O, 512], F32, tag="h1_ps", name=f"h1_{c}")
                nc.tensor.matmul(h1_ps[:, :], w1_sb[:, :], h_sb[:, sl],
                                 start=True, stop=True)
                aggr_tiles[c + 100] = h1_ps  # stash for part b

            def post_slab_b(c):
                """h1 -> SBUF (+sum) and Square (+sumsq) BN partials."""
                sl = slice(c * 512, (c + 1) * 512)
                h1_ps = aggr_tiles[c + 100]
                nc.scalar.activation(h1_sb[:, sl], h1_ps[:, :], AF.Copy,
                                     accum_out=stats_s[:, c:c + 1])
                nc.scalar.activation(sq_scr[:, sl], h1_sb[:, sl], AF.Square,
                                     accum_out=stats_q[:, c:c + 1])

            # ---- emission: scan-paced interleave ------------------------
            def make_aggr(c):
                aggr_tiles[c] = ps_a.tile([O, 512], F32, tag="aggr", name=f"ag{c}")

            # stripes 0..3 first (tau chunk 0), then per round: one stripe's
            # 8 s-pieces 1:1-interleaved with 8 phase-2 units (slab c's units
            # land in rounds 4c..4c+3 by construction). post-slab work is
            # emitted 1-2 rounds late so its cross-engine round trips never
            # head-block the in-order ACT/Pool queues that feed the masks.
            for j in range(4):
                for c in range(NCH):
                    s_piece(j, c)
                stripe_merge(j)
            tau_load(0)
            uq = [(c, j) for c in range(NCH - 1) for j in range(NT)]
            deferred = []
            pend = []       # signed masks whose (lagged) aggr-mm is not yet emitted
            LAG = 3

            def drain_one(t):
                cjm = pend.pop(0)
                if cjm[1] == 0:
                    make_aggr(cjm[0])
                unit_a(cjm)
                if cjm[1] == NT - 1:
                    cc = cjm[0]
                    deferred.append((t + 1, (lambda c_: lambda: post_slab_a(c_))(cc)))
                    deferred.append((t + 2, (lambda c_: lambda: post_slab_b(c_))(cc)))

            for t in range(28):
                due = [fn for (r, fn) in deferred if r <= t]
                deferred = [(r, fn) for (r, fn) in deferred if r > t]
                for fn in due:
                    fn()
                units = uq[t * 8:(t + 1) * 8]
                for i in range(8):
                    pend.append(unit_v(*units[i]))
                    if len(pend) > LAG:
                        drain_one(t)
                    s_piece(4 + t, i)
                stripe_merge(4 + t)
                if t % 4 == 3 and t // 4 + 1 < NCH:
                    tau_load(t // 4 + 1)
            while pend:
                drain_one(28)
            for (r, fn) in deferred:
                fn()
            for j in range(NT):
                pend.append(unit_v(NCH - 1, j))
                if len(pend) > LAG:
                    drain_one(28)
            while pend:
                drain_one(28)
            post_slab_a(NCH - 1)
            post_slab_b(NCH - 1)

            # ---- BN combine + AllReduce + GELU + W2 ---------------------
            stats = work.tile([O, 2], F32)
            nc.vector.reduce_sum(stats[:, 0:1], stats_s[:, :],
                                 axis=mybir.AxisListType.X)
            nc.vector.reduce_sum(stats[:, 1:2], stats_q[:, :],
                                 axis=mybir.AxisListType.X)

            cc_in = dpool.tile([O, 2], F32)
            cc_out = dpool.tile([O, 2], F32, addr_space="Shared")
            nc.sync.dma_start(cc_in[:, :], stats[:, :])
            nc.gpsimd.collective_compute(
                "AllReduce", ALU.add,
                ins=[cc_in[:, :]],
                outs=[cc_out[:, :]],
                replica_groups=[list(range(N_CORES))],
            )
            gstats = work.tile([O, 2], F32)
            nc.sync.dma_start(gstats[:, :], cc_out[:, :])

            mean = work.tile([O, 1], F32)
            var = work.tile([O, 1], F32)
            scale = work.tile([O, 1], F32)
            shift = work.tile([O, 1], F32)
            tmp = work.tile([O, 1], F32)
            nc.vector.tensor_scalar(out=mean[:, :], in0=gstats[:, 0:1],
                                    scalar1=1.0 / BN_COUNT, scalar2=None,
                                    op0=ALU.mult)
            nc.vector.tensor_scalar(out=var[:, :], in0=gstats[:, 1:2],
                                    scalar1=1.0 / BN_COUNT, scalar2=None,
                                    op0=ALU.mult)
            nc.vector.tensor_tensor(out=tmp[:, :], in0=mean[:, :], in1=mean[:, :],
                                    op=ALU.mult)
            nc.vector.tensor_tensor(out=var[:, :], in0=var[:, :], in1=tmp[:, :],
                                    op=ALU.subtract)
            nc.vector.tensor_scalar(out=var[:, :], in0=var[:, :], scalar1=BN_EPS,
                                    scalar2=None, op0=ALU.add)
            nc.scalar.activation(tmp[:, :], var[:, :], AF.Sqrt)
            nc.vector.reciprocal(out=tmp[:, :], in_=tmp[:, :])
            nc.vector.tensor_tensor(out=scale[:, :], in0=vecs_sb[:, 0:1],
                                    in1=tmp[:, :], op=ALU.mult)
            nc.vector.tensor_tensor(out=tmp[:, :], in0=mean[:, :], in1=scale[:, :],
                                    op=ALU.mult)
            nc.vector.tensor_tensor(out=shift[:, :], in0=vecs_sb[:, 1:2],
                                    in1=tmp[:, :], op=ALU.subtract)

            for c in range(NCH):
                sl = slice(c * 512, (c + 1) * 512)
                nc.scalar.activation(hg_sb[:, sl], h1_sb[:, sl], AF.Gelu,
                                     scale=scale[:, :], bias=shift[:, :])
                o_ps = ps_m.tile([O, 512], F32, tag="h1_ps", name=f"o_{c}")
                nc.tensor.matmul(o_ps[:, :], w2_sb[:, :], hg_sb[:, sl],
                                 start=True, stop=True)
                nc.vector.tensor_scalar(out=y_sb[:, sl], in0=o_ps[:, :],
                                        scalar1=vecs_sb[:, 2:3], scalar2=None,
                                        op0=ALU.add)
                nc.sync.dma_start(y_d[:, sl], y_sb[:, sl])

            for cm in reversed(_cms):
                cm.__exit__(None, None, None)

    if not nc.is_finalized():
        nc.finalize()
    return nc


def _get_runner():
    """Build once; cache a jitted 8-core shard_map executable."""
    if "runner" in _cache:
        return _cache["runner"]

    import jax
    import concourse.mybir as mb
    from jax.sharding import Mesh, PartitionSpec
    from jax.experimental.shard_map import shard_map
    from concourse import bass2jax

    nc = _build()
    bass2jax.install_neuronx_cc_hook()

    partition_name = nc.partition_id_tensor.name if nc.partition_id_tensor else None
    in_names = []
    out_names = []
    out_avals = []
    for alloc in nc.m.functions[0].allocations:
        if not isinstance(alloc, mb.MemoryLocationSet):
            continue
        name = alloc.memorylocations[0].name
        if alloc.kind == "ExternalInput":
            if name != partition_name:
                in_names.append(name)
        elif alloc.kind == "ExternalOutput":
            out_names.append(name)
            out_avals.append(
                jax.core.ShapedArray(tuple(alloc.tensor_shape), mb.dt.np(alloc.dtype))
            )
    n_params = len(in_names)
    all_in_names = list(in_names)
    if partition_name is not None:
        all_in_names = all_in_names + [partition_name]

    def _body(*args):
        operands = list(args)
        if partition_name is not None:
            operands.append(bass2jax.partition_id_tensor())
        outs = bass2jax._bass_exec_p.bind(
            *operands,
            out_avals=tuple(out_avals),
            in_names=tuple(all_in_names),
            out_names=tuple(out_names),
            lowering_input_output_aliases=(),
            sim_require_finite=True,
            sim_require_nnan=True,
            nc=nc,
        )
        return tuple(outs)

    devices = jax.devices()[:N_CORES]
    assert len(devices) == N_CORES, f"need {N_CORES} devices, have {len(jax.devices())}"
    mesh = Mesh(np.asarray(devices), ("core",))
    n_outs = len(out_names)
    sharded = jax.jit(
        shard_map(
            _body,
            mesh=mesh,
            in_specs=(PartitionSpec("core"),) * n_params,
            out_specs=(PartitionSpec("core"),) * n_outs,
            check_rep=False,
        ),
        keep_unused=True,
    )
    _cache["runner"] = (sharded, in_names, out_names, out_avals)
    return _cache["runner"]


def kernel(**inputs) -> np.ndarray:
    x = np.asarray(inputs["x"], dtype=np.float32)
    assert x.shape == (B, C, N, 1), x.shape
    k = int(np.asarray(inputs.get("k", K_NN)))
    assert k == K_NN, f"kernel compiled for k={K_NN}, got {k}"
    w1 = np.asarray(inputs["w1"], dtype=np.float32)
    b1 = np.asarray(inputs["b1"], dtype=np.float32)  # cancels through BN stats
    gamma = np.asarray(inputs["gamma"], dtype=np.float32)
    beta = np.asarray(inputs["beta"], dtype=np.float32)
    w2 = np.asarray(inputs["w2"], dtype=np.float32)
    b2 = np.asarray(inputs["b2"], dtype=np.float32)
    eps_gin = float(np.asarray(inputs["eps_gin"]))
    del b1

    sharded, in_names, out_names, out_avals = _get_runner()

    xb = np.ascontiguousarray(x[:, :, :, 0])                 # [B, C, N]
    hi = _f32r_round(xb)                                     # [B, C, N]
    sq = (xb.astype(np.float64) ** 2).sum(axis=1)            # [B, N]
    q_hi = _f32r_round((-0.5 * sq).astype(np.float32))
    q_lo = _f32r_round((-0.5 * sq - q_hi.astype(np.float64)).astype(np.float32))

    xh1 = np.empty((B, 66, N), np.float32)
    xh1[:, :C] = hi
    xh1[:, C] = 1.0
    xh1[:, C + 1] = 1.0
    xh2 = np.empty((B, 68, N), np.float32)
    xh2[:, :C] = hi
    xh2[:, C] = q_hi
    xh2[:, C + 1] = q_lo
    xh2[:, C + 2] = -1.0
    xh2[:, C + 3] = -1.0

    xt16 = xb.astype(ml_dtypes.bfloat16)                     # [B, C, N]
    # xt[p, j*C + c] = 0.5 * bf16(x[c, j*128 + p])  (halved exactly, so the
    # +-1 sign-mask aggregation lands as 0.5*S_sign in PSUM)
    xt_half = (xt16.astype(np.float32) * 0.5).astype(ml_dtypes.bfloat16)
    xt = np.ascontiguousarray(
        xt_half.reshape(B, C, NT, 128).transpose(0, 3, 2, 1).reshape(B, 128, NT * C))
    rowsum = xt16.astype(np.float64).sum(axis=2)             # [B, C]
    xeps = ((1.0 + eps_gin) * xb.astype(np.float64)
            + 0.5 * rowsum[:, :, None]).astype(np.float32)   # [B, C, N]

    vecs = np.stack([gamma, beta, b2], axis=1).astype(np.float32)
    per_core = {
        "xh1": xh1,
        "xh2": xh2,
        "xt": xt,
        "xeps": xeps,
        "w1r": np.broadcast_to(w1, (N_CORES,) + w1.shape),
        "w2r": np.broadcast_to(_f32r_round(w2), (N_CORES,) + w2.shape),
        "vecs": np.broadcast_to(vecs, (N_CORES,) + vecs.shape),
    }
    concat_in = [
        np.ascontiguousarray(per_core[name]).reshape(
            (N_CORES * per_core[name].shape[1],) + per_core[name].shape[2:]
        )
        for name in in_names
    ]
    out_arrs = sharded(*concat_in)
    yi = out_names.index("y")
    y = np.asarray(out_arrs[yi]).reshape(N_CORES, O, N)
    return y[..., None].astype(np.float32)


# revision 29
# speedup vs baseline: 1.9373x; 1.0409x over previous
"""DyGraphGIN2d Trainium kernel: kNN graph (k=16) + GIN aggregation + MLP/BN/GELU.

Sharding: data-parallel over batch B=8 across 8 NeuronCores (one element per
core). BatchNorm batch statistics are combined with one small AllReduce.

Per-core algorithm (N=4096 nodes, C=64 channels). All static operand prep is
done HOST-side in numpy (f32r rounding, q = -|x|^2/2 split, bf16 transposed x,
(1+eps)x + rowsum/2) so the device runs only matmuls + scan + masks:

  Phase 1 (threshold): ranking value s[n,m] = <hi_n,hi_m> + q_hi_m + q_lo_m
    via ONE f32r matmul per [128,512] tile (66-row contraction: 64 hi rows +
    two ones rows picking up the exact q split; matmul cost depends only on
    columns). f32r operand rounding adds ~2.5e-3 noise to s, which flips the
    16/17-neighbor choice on ~0.3% of rows (measured end-to-end 5.7e-3 rel
    err vs the 2e-2 budget). DVE top-8 per 512-chunk -> 64 candidates ->
    max/match_replace/max gives each row's 16th-largest tau exactly.
  Phase 2 (mask+aggregate): v'[m,n] = s[n,m] - tau[n] recomputed in the
    transposed orientation with the same 66-row matmul plus a 67th row
    (-1 stationary x tau moving), bit-identical to phase 1 up to the final
    tau subtraction (guard 5e-5 covers its rounding). mask = Sign(v') on the
    ACT engine (+-1 exact in bf16, straight from PSUM - no DVE pass).
    aggr = xt^T @ mask accumulates 0.5*(sum_sel - sum_unsel) in PSUM;
    h = 0.5*aggr + [(1+eps)x + 0.5*rowsum] (host-prepped Xeps) on GPSIMD.
  Pipeline: column-slab c (512 n-cols) only needs tau from stripes 4c..4c+3,
    so mask/aggregate work for early slabs overlaps the DVE scan of later
    stripes (the scan, ~190us, is the pacing engine).
  Tail: h1 = w1^T h; BN stats sum/sumsq per slab (ACT accum) -> AllReduce
    -> fused BN+erf-GELU -> w2 -> y.
"""

import numpy as np
import ml_dtypes

import concourse.bacc as bacc
import concourse.mybir as mybir
from concourse.tile import TileContext

F32 = mybir.dt.float32
F32R = mybir.dt.float32r
BF16 = mybir.dt.bfloat16
AF = mybir.ActivationFunctionType
ALU = mybir.AluOpType

B, C, N, O = 8, 64, 4096, 64
K_NN = 16
N_CORES = 8
NT = N // 128            # 32 row stripes
NCH = N // 512           # 8 column chunks / slabs
BN_EPS = 1e-5
BN_COUNT = float(B * N)
TAU_GUARD = 5e-5

_cache = {}


def _f32r_round(a):
    """Round fp32 to 11 explicit mantissa bits (matches f32r storage)."""
    a = np.ascontiguousarray(a, np.float32)
    bits = a.view(np.uint32).astype(np.uint64)
    shift = 23 - 11
    half = np.uint64(1 << (shift - 1))
    mask = np.uint64(~((1 << shift) - 1) & 0xFFFFFFFF)
    return ((bits + half) & mask).astype(np.uint32).view(np.float32)


def _build():
    nc = bacc.Bacc("TRN2", target_bir_lowering=False)

    # host-prepped operands
    xh1_d = nc.dram_tensor("xh1", [66, N], F32R, kind="ExternalInput")   # hi;1;1
    xh2_d = nc.dram_tensor("xh2", [68, N], F32R, kind="ExternalInput")   # hi;qh;ql;-1;-1
    xt_d = nc.dram_tensor("xt", [128, NT * C], BF16, kind="ExternalInput")
    xtf_d = nc.dram_tensor("xtf", [128, NT * C], BF16, kind="ExternalInput")
    xeps_d = nc.dram_tensor("xeps", [C, N], F32, kind="ExternalInput")   # (1+e)x+rs/2
    w1_d = nc.dram_tensor("w1r", [C, O], F32, kind="ExternalInput")
    w2_d = nc.dram_tensor("w2r", [O + 1, O], F32R, kind="ExternalInput")  # w2;b2
    vecs_d = nc.dram_tensor("vecs", [O, 3], F32, kind="ExternalInput")   # gamma,beta,b2
    y_d = nc.dram_tensor("y", [O, N], F32, kind="ExternalOutput")
    tau_dram = nc.dram_tensor("tau_scratch", [N, 2], F32)                # internal

    with TileContext(nc) as tc:
        with tc.tile_pool(name="big", bufs=1) as big, \
             tc.tile_pool(name="work", bufs=1) as work, \
             tc.tile_pool(name="dram", bufs=1, space="DRAM") as dpool:

            # ---- inputs -> SBUF (chunked for fine-grained deps) ---------
            xh1c = [big.tile([68, 512], F32R, name=f"xh1c{i}") for i in range(NCH)]
            xh2c = [big.tile([68, 512], F32R, name=f"xh2c{i}") for i in range(NCH)]
            for i in range(NCH):
                sl = slice(i * 512, (i + 1) * 512)
                nc.sync.dma_start(xh1c[i][0:66, :], xh1_d[:, sl])
                nc.sync.dma_start(xh2c[i][:, :], xh2_d[:, sl])
            xt_sb = big.tile([128, NT * C], BF16)
            nc.sync.dma_start(xt_sb[:, :], xt_d[:, :])
            xtf_sb = big.tile([128, NT * C], BF16)
            nc.sync.dma_start(xtf_sb[:, :], xtf_d[:, :])
            xeps_sb = big.tile([C, N], F32)
            nc.sync.dma_start(xeps_sb[:, :], xeps_d[:, :])
            w1_sb = work.tile([C, O], F32)
            w2_sb = work.tile([O + 1, O], F32R)
            vecs_sb = work.tile([O, 3], F32)
            nc.sync.dma_start(w1_sb[:, :], w1_d[:, :])
            nc.sync.dma_start(w2_sb[:, :], w2_d[:, :])
            nc.sync.dma_start(vecs_sb[:, :], vecs_d[:, :])

            h_sb = big.tile([C, N], F32)
            y_sb = big.tile([C, N], F32)
            h1_sb = big.tile([C, N], F32)
            sq_scr = big.tile([C, N], F32)
            hg_sb = big.tile([C + 1, N], F32R)   # row 64 = ones (b2 via matmul)
            nc.gpsimd.memset(hg_sb.bitcast(F32)[C:C + 1, :], 1.0)
            stats_s = work.tile([O, NCH], F32)   # per-slab sum(h1)
            stats_q = work.tile([O, NCH], F32)   # per-slab sum(h1^2)

            cand = work.tile([128, 64], F32, tag="cand", bufs=3)
            t8a = work.tile([128, 8], F32, tag="t8a", bufs=3)
            t8b = work.tile([128, 8], F32, tag="t8b", bufs=3)
            tau_f = work.tile([128, 1], F32, tag="tauf", bufs=3)
            tau2 = work.tile([128, 2], F32R, tag="tau2", bufs=3)
            NMASK = 8
            mask_ring = [work.tile([128, 512], BF16, name=f"mask{i}")
                         for i in range(NMASK)]

            ps_s_cm = tc.tile_pool(name="ps_s", bufs=2, space="PSUM")
            ps_v_cm = tc.tile_pool(name="ps_v", bufs=3, space="PSUM")
            ps_a_cm = tc.tile_pool(name="ps_a", bufs=2, space="PSUM")
            ps_m_cm = tc.tile_pool(name="ps_m", bufs=1, space="PSUM")
            ps_s = ps_s_cm.__enter__()
            ps_v = ps_v_cm.__enter__()
            ps_a = ps_a_cm.__enter__()
            ps_m = ps_m_cm.__enter__()
            _cms = [ps_s_cm, ps_v_cm, ps_a_cm, ps_m_cm]

            aggr_tiles = {}

            def s_piece(j, c):
                """One phase-1 s-matmul chunk + its DVE top-8."""
                jt, jo = j // 4, (j % 4) * 128
                s_ps = ps_s.tile([128, 512], F32, tag="s_ps", name=f"s_{j}_{c}")
                nc.tensor.matmul(s_ps[:, :], xh1c[jt][0:66, jo:jo + 128],
                                 xh2c[c][0:66, :], start=True, stop=True)
                nc.vector.max(out=cand[:, c * 8:(c + 1) * 8], in_=s_ps[:, :])

            def stripe_merge(j):
                """DVE merge of the 64 candidates -> tau hi/lo -> DMA."""
                nc.vector.max(out=t8a[:, :], in_=cand[:, :])
                nc.vector.match_replace(out=cand[:, :], in_to_replace=t8a[:, :],
                                        in_values=cand[:, :], imm_value=-1e30)
                nc.vector.max(out=t8b[:, :], in_=cand[:, :])
                # tau = t16 - guard, split exactly into f32r hi + lo rows so
                # the phase-2 matmul subtracts it at full fp32 precision.
                nc.vector.tensor_scalar(out=tau_f[:, :], in0=t8b[:, 7:8],
                                        scalar1=TAU_GUARD, scalar2=None,
                                        op0=ALU.subtract)
                nc.vector.tensor_copy(tau2[:, 0:1], tau_f[:, :])
                nc.gpsimd.tensor_tensor(out=tau2.bitcast(F32)[:, 1:2],
                                        in0=tau_f[:, :],
                                        in1=tau2.bitcast(F32)[:, 0:1],
                                        op=ALU.subtract)
                nc.sync.dma_start(tau_dram[j * 128:(j + 1) * 128, :],
                                  tau2.bitcast(F32)[:, :])

            def tau_load(c):
                """tau hi/lo rows for slab c into xh1c[c] partitions 66/67."""
                for r in range(2):
                    nc.sync.dma_start(
                        xh1c[c].bitcast(F32)[66 + r:67 + r, :],
                        tau_dram[c * 512:(c + 1) * 512, r:r + 1]
                        .rearrange("m one -> one m"))

            _ucount = [0]

            def unit_v(c, j, on_dve=False):
                """Phase-2: v'[stripe j, slab c] matmul -> mask.

                ACT path: Sign -> +-1 mask, aggregated against the halved xt.
                DVE path ((v'>=0)-0.5 -> +-0.5) against the full-scale xt;
                used where DVE has idle capacity (after its scan ends)."""
                g = _ucount[0]
                _ucount[0] += 1
                jt, jo = j // 4, (j % 4) * 128
                v_ps = ps_v.tile([128, 512], F32, tag="v_ps", name=f"v_{c}_{j}")
                nc.tensor.matmul(v_ps[:, :], xh2c[jt][0:68, jo:jo + 128],
                                 xh1c[c][0:68, :], start=True, stop=True)
                mt = mask_ring[g % NMASK]
                if on_dve:
                    nc.vector.tensor_scalar(out=mt[:, :], in0=v_ps[:, :],
                                            scalar1=0.0, scalar2=0.5,
                                            op0=ALU.is_ge, op1=ALU.subtract)
                else:
                    nc.scalar.activation(mt[:, :], v_ps[:, :], AF.Sign)
                return (c, j, mt, on_dve)

            def unit_a(cjm):
                """Lagged aggregation matmul for a completed mask."""
                c, j, mt, on_dve = cjm
                xs = xtf_sb if on_dve else xt_sb
                nc.tensor.matmul(aggr_tiles[c][:, :],
                                 xs[:, j * C:(j + 1) * C], mt[:, :],
                                 start=(j == 0), stop=(j == NT - 1))

            aggr_sb = work.tile([O, 512], F32, tag="aggr_sb", bufs=2)

            def post_slab_a(c):
                """h = 0.5*aggr + Xeps (ACT copy out of PSUM, Pool add), w1."""
                sl = slice(c * 512, (c + 1) * 512)
                nc.scalar.activation(aggr_sb[:, :], aggr_tiles[c][:, :], AF.Copy)
                nc.gpsimd.tensor_tensor(out=h_sb[:, sl], in0=aggr_sb[:, :],
                                        in1=xeps_sb[:, sl], op=ALU.add)
                h1_ps = ps_m.tile([O, 512], F32, tag="h1_ps", name=f"h1_{c}")
                nc.tensor.matmul(h1_ps[:, :], w1_sb[:, :], h_sb[:, sl],
                                 start=True, stop=True)
                aggr_tiles[c + 100] = h1_ps  # stash for part b

            def post_slab_b(c):
                """h1 -> SBUF (+sum) and Square (+sumsq) BN partials."""
                sl = slice(c * 512, (c + 1) * 512)
                h1_ps = aggr_tiles[c + 100]
                nc.scalar.activation(h1_sb[:, sl], h1_ps[:, :], AF.Copy,
                                     accum_out=stats_s[:, c:c + 1])
                nc.scalar.activation(sq_scr[:, sl], h1_sb[:, sl], AF.Square,
                                     accum_out=stats_q[:, c:c + 1])

            # ---- emission: scan-paced interleave ------------------------
            def make_aggr(c):
                aggr_tiles[c] = ps_a.tile([O, 512], F32, tag="aggr", name=f"ag{c}")

            # stripes 0..3 first (tau chunk 0), then per round: one stripe's
            # 8 s-pieces 1:1-interleaved with 8 phase-2 units (slab c's units
            # land in rounds 4c..4c+3 by construction). post-slab work is
            # emitted 1-2 rounds late so its cross-engine round trips never
            # head-block the in-order ACT/Pool queues that feed the masks.
            for j in range(4):
                for c in range(NCH):
                    s_piece(j, c)
                stripe_merge(j)
            tau_load(0)
            uq = [(c, j) for c in range(NCH - 1) for j in range(NT)]
            deferred = []
            pend = []       # signed masks whose (lagged) aggr-mm is not yet emitted
            LAG = 3

            def drain_one(t):
                cjm = pend.pop(0)
                if cjm[1] == 0:
                    make_aggr(cjm[0])
                unit_a(cjm)
                if cjm[1] == NT - 1:
                    cc = cjm[0]
                    deferred.append((t + 1, (lambda c_: lambda: post_slab_a(c_))(cc)))
                    deferred.append((t + 2, (lambda c_: lambda: post_slab_b(c_))(cc)))

            for t in range(28):
                due = [fn for (r, fn) in deferred if r <= t]
                deferred = [(r, fn) for (r, fn) in deferred if r > t]
                for fn in due:
                    fn()
                units = uq[t * 8:(t + 1) * 8]
                for i in range(8):
                    pend.append(unit_v(*units[i]))
                    if len(pend) > LAG:
                        drain_one(t)
                    s_piece(4 + t, i)
                stripe_merge(4 + t)
                if t % 4 == 3 and t // 4 + 1 < NCH:
                    tau_load(t // 4 + 1)
            while pend:
                drain_one(28)
            for (r, fn) in deferred:
                fn()
            # last slab: DVE is done scanning, so alternate masks ACT/DVE
            for j in range(NT):
                pend.append(unit_v(NCH - 1, j, on_dve=(j % 2 == 1)))
                if len(pend) > LAG:
                    drain_one(28)
            while pend:
                drain_one(28)
            post_slab_a(NCH - 1)
            post_slab_b(NCH - 1)

            # ---- BN combine + AllReduce + GELU + W2 ---------------------
            stats = work.tile([O, 2], F32)
            nc.vector.reduce_sum(stats[:, 0:1], stats_s[:, :],
                                 axis=mybir.AxisListType.X)
            nc.vector.reduce_sum(stats[:, 1:2], stats_q[:, :],
                                 axis=mybir.AxisListType.X)

            # AllGather (15us flat) beats AllReduce (28us) in the collective
            # cost model; the 8-way sum is 7 trivial DVE adds.
            cc_in = dpool.tile([O, 2], F32)
            cc_out = dpool.tile([N_CORES * O, 2], F32, addr_space="Shared")
            nc.sync.dma_start(cc_in[:, :], stats[:, :])
            nc.gpsimd.collective_compute(
                "AllGather", ALU.bypass,
                ins=[cc_in[:, :]],
                outs=[cc_out[:, :]],
                replica_groups=[list(range(N_CORES))],
            )
            gs_all = work.tile([O, 2 * N_CORES], F32)
            for kk in range(N_CORES):
                nc.sync.dma_start(gs_all[:, 2 * kk:2 * kk + 2],
                                  cc_out[kk * O:(kk + 1) * O, :])
            gstats = work.tile([O, 2], F32)
            nc.vector.tensor_tensor(out=gstats[:, :], in0=gs_all[:, 0:2],
                                    in1=gs_all[:, 2:4], op=ALU.add)
            for kk in range(2, N_CORES):
                nc.vector.tensor_tensor(out=gstats[:, :], in0=gstats[:, :],
                                        in1=gs_all[:, 2 * kk:2 * kk + 2],
                                        op=ALU.add)

            mean = work.tile([O, 1], F32)
            var = work.tile([O, 1], F32)
            scale = work.tile([O, 1], F32)
            shift = work.tile([O, 1], F32)
            tmp = work.tile([O, 1], F32)
            nc.vector.tensor_scalar(out=mean[:, :], in0=gstats[:, 0:1],
                                    scalar1=1.0 / BN_COUNT, scalar2=None,
                                    op0=ALU.mult)
            nc.vector.tensor_scalar(out=var[:, :], in0=gstats[:, 1:2],
                                    scalar1=1.0 / BN_COUNT, scalar2=None,
                                    op0=ALU.mult)
            nc.vector.tensor_tensor(out=tmp[:, :], in0=mean[:, :], in1=mean[:, :],
                                    op=ALU.mult)
            nc.vector.tensor_tensor(out=var[:, :], in0=var[:, :], in1=tmp[:, :],
                                    op=ALU.subtract)
            nc.vector.tensor_scalar(out=var[:, :], in0=var[:, :], scalar1=BN_EPS,
                                    scalar2=None, op0=ALU.add)
            nc.scalar.activation(tmp[:, :], var[:, :], AF.Sqrt)
            nc.vector.reciprocal(out=tmp[:, :], in_=tmp[:, :])
            nc.vector.tensor_tensor(out=scale[:, :], in0=vecs_sb[:, 0:1],
                                    in1=tmp[:, :], op=ALU.mult)
            nc.vector.tensor_tensor(out=tmp[:, :], in0=mean[:, :], in1=scale[:, :],
                                    op=ALU.mult)
            nc.vector.tensor_tensor(out=shift[:, :], in0=vecs_sb[:, 1:2],
                                    in1=tmp[:, :], op=ALU.subtract)

            for c in range(NCH):
                sl = slice(c * 512, (c + 1) * 512)
                nc.scalar.activation(hg_sb[0:C, sl], h1_sb[:, sl], AF.Gelu,
                                     scale=scale[:, :], bias=shift[:, :])
                o_ps = ps_m.tile([O, 512], F32, tag="h1_ps", name=f"o_{c}")
                nc.tensor.matmul(o_ps[:, :], w2_sb[:, :], hg_sb[0:C + 1, sl],
                                 start=True, stop=True)
                nc.scalar.activation(y_sb[:, sl], o_ps[:, :], AF.Copy)
                nc.sync.dma_start(y_d[:, sl], y_sb[:, sl])

            for cm in reversed(_cms):
                cm.__exit__(None, None, None)

    if not nc.is_finalized():
        nc.finalize()
    return nc


def _get_runner():
    """Build once; cache a jitted 8-core shard_map executable."""
    if "runner" in _cache:
        return _cache["runner"]

    import jax
    import concourse.mybir as mb
    from jax.sharding import Mesh, PartitionSpec
    from jax.experimental.shard_map import shard_map
    from concourse import bass2jax

    nc = _build()
    bass2jax.install_neuronx_cc_hook()

    partition_name = nc.partition_id_tensor.name if nc.partition_id_tensor else None
    in_names = []
    out_names = []
    out_avals = []
    for alloc in nc.m.functions[0].allocations:
        if not isinstance(alloc, mb.MemoryLocationSet):
            continue
        name = alloc.memorylocations[0].name
        if alloc.kind == "ExternalInput":
            if name != partition_name:
                in_names.append(name)
        elif alloc.kind == "ExternalOutput":
            out_names.append(name)
            out_avals.append(
                jax.core.ShapedArray(tuple(alloc.tensor_shape), mb.dt.np(alloc.dtype))
            )
    n_params = len(in_names)
    all_in_names = list(in_names)
    if partition_name is not None:
        all_in_names = all_in_names + [partition_name]

    def _body(*args):
        operands = list(args)
        if partition_name is not None:
            operands.append(bass2jax.partition_id_tensor())
        outs = bass2jax._bass_exec_p.bind(
            *operands,
            out_avals=tuple(out_avals),
            in_names=tuple(all_in_names),
            out_names=tuple(out_names),
            lowering_input_output_aliases=(),
            sim_require_finite=True,
            sim_require_nnan=True,
            nc=nc,
        )
        return tuple(outs)

    devices = jax.devices()[:N_CORES]
    assert len(devices) == N_CORES, f"need {N_CORES} devices, have {len(jax.devices())}"
    mesh = Mesh(np.asarray(devices), ("core",))
    n_outs = len(out_names)
    sharded = jax.jit(
        shard_map(
            _body,
            mesh=mesh,
            in_specs=(PartitionSpec("core"),) * n_params,
            out_specs=(PartitionSpec("core"),) * n_outs,
            check_rep=False,
        ),
        keep_unused=True,
    )
    _cache["runner"] = (sharded, in_names, out_names, out_avals)
    return _cache["runner"]


def kernel(**inputs) -> np.ndarray:
    x = np.asarray(inputs["x"], dtype=np.float32)
    assert x.shape == (B, C, N, 1), x.shape
    k = int(np.asarray(inputs.get("k", K_NN)))
    assert k == K_NN, f"kernel compiled for k={K_NN}, got {k}"
    w1 = np.asarray(inputs["w1"], dtype=np.float32)
    b1 = np.asarray(inputs["b1"], dtype=np.float32)  # cancels through BN stats
    gamma = np.asarray(inputs["gamma"], dtype=np.float32)
    beta = np.asarray(inputs["beta"], dtype=np.float32)
    w2 = np.asarray(inputs["w2"], dtype=np.float32)
    b2 = np.asarray(inputs["b2"], dtype=np.float32)
    eps_gin = float(np.asarray(inputs["eps_gin"]))
    del b1

    sharded, in_names, out_names, out_avals = _get_runner()

    xb = np.ascontiguousarray(x[:, :, :, 0])                 # [B, C, N]
    hi = _f32r_round(xb)                                     # [B, C, N]
    sq = (xb.astype(np.float64) ** 2).sum(axis=1)            # [B, N]
    q_hi = _f32r_round((-0.5 * sq).astype(np.float32))
    q_lo = _f32r_round((-0.5 * sq - q_hi.astype(np.float64)).astype(np.float32))

    xh1 = np.empty((B, 66, N), np.float32)
    xh1[:, :C] = hi
    xh1[:, C] = 1.0
    xh1[:, C + 1] = 1.0
    xh2 = np.empty((B, 68, N), np.float32)
    xh2[:, :C] = hi
    xh2[:, C] = q_hi
    xh2[:, C + 1] = q_lo
    xh2[:, C + 2] = -1.0
    xh2[:, C + 3] = -1.0

    xt16 = xb.astype(ml_dtypes.bfloat16)                     # [B, C, N]
    # xt[p, j*C + c] = 0.5 * bf16(x[c, j*128 + p])  (halved exactly, so the
    # +-1 sign-mask aggregation lands as 0.5*S_sign in PSUM)
    xt_half = (xt16.astype(np.float32) * 0.5).astype(ml_dtypes.bfloat16)
    xt = np.ascontiguousarray(
        xt_half.reshape(B, C, NT, 128).transpose(0, 3, 2, 1).reshape(B, 128, NT * C))
    xtf = np.ascontiguousarray(
        xt16.reshape(B, C, NT, 128).transpose(0, 3, 2, 1).reshape(B, 128, NT * C))
    rowsum = xt16.astype(np.float64).sum(axis=2)             # [B, C]
    xeps = ((1.0 + eps_gin) * xb.astype(np.float64)
            + 0.5 * rowsum[:, :, None]).astype(np.float32)   # [B, C, N]

    vecs = np.stack([gamma, beta, b2], axis=1).astype(np.float32)
    per_core = {
        "xh1": xh1,
        "xh2": xh2,
        "xt": xt,
        "xtf": xtf,
        "xeps": xeps,
        "w1r": np.broadcast_to(w1, (N_CORES,) + w1.shape),
        "w2r": np.broadcast_to(
            _f32r_round(np.concatenate([w2, b2[None, :]], axis=0)),
            (N_CORES, O + 1, O)),
        "vecs": np.broadcast_to(vecs, (N_CORES,) + vecs.shape),
    }
    concat_in = [
        np.ascontiguousarray(per_core[name]).reshape(
            (N_CORES * per_core[name].shape[1],) + per_core[name].shape[2:]
        )
        for name in in_names
    ]
    out_arrs = sharded(*concat_in)
    yi = out_names.index("y")
    y = np.asarray(out_arrs[yi]).reshape(N_CORES, O, N)
    return y[..., None].astype(np.float32)
